# revision 14
# baseline (speedup 1.0000x reference)
"""Trainium2 Bass kernel for nn_CubicModelLarge (3-layer cubic-feature MLP).

v3: output-sharded tensor parallelism in fp16, AllGather boundaries,
DMA-count-minimized rotation construction.

Per layer, each core owns a contiguous block of 8 outputs (layer 2: 10
outputs padded to 16, 2 per core, host concatenates).  The cubic expansion is
never materialized; per core:

  H[(o,i), b] = sum_J Wcub[J, (o,i)] * F[J, b]      (17x 128-row GEMM chunks)
  y^T[o, b]   = lin[o, b] + sum_i x[b,i] * H[(o,i), b]

F rows (2176 = 17*128): 2048 rotation products x_a*x_{(a+d)%64} (d=0..31),
64 x rows (carry the symmetrized W_sq fold), 64 gap-32 products (halved).

Rotations: xw8[p, s] = x_{(p+s)%64} (8 shifted copies, built with 10 DMAs via
doubling shifts) lets one rearranged SBUF->SBUF DMA emit a (128, 4, HB) tile
holding 4 chunks' rotated operands: rot[(q a), j, c] = xw8[2k0+a, 2j+q, c].
Products run on DVE in fp16 2x mode (all-SBUF).  The i-contraction is an
elementwise multiply by [x;x] (DVE) plus 0/1 segment-sum matmuls on PE
accumulating into y^T PSUM along with the linear term.

Each core computes the full i-range, so result slices are complete: the layer
boundary is an AllGather of y^T (8, 512) fp16 per batch-half whose output
(64, 512) IS the next half's xT (x is fed in pre-transposed).  Batch halves
pipeline: half 0's AllGather hides under half 1's GEMMs.

HWDGE dma_start costs ~600ns of sequencer time each, so half 0 traffic is
issued on the SP ring and half 1 + weights on the ACT ring.
"""

import numpy as np

D = 64
B = 1024
NCORES = 8
OUTS = (64, 64, 10)
OPER = (8, 8, 2)             # outputs per core (layer 2 padded to 16)
NKCHUNK = 16                 # rotation chunks (d pairs)
NCHUNK = 17                  # + the [x; d32] chunk
HB = 512                     # half-batch

_CACHE = {}


# ---------------------------------------------------------------- host prep --

def _maps():
    iu, ju = np.triu_indices(D)
    tmap = np.zeros((D, D), np.int64)
    tmap[iu, ju] = np.arange(len(iu))
    tmap[ju, iu] = tmap[iu, ju]
    p = np.arange(128)
    rows_t = np.zeros((NKCHUNK, 128), np.int64)
    for k in range(NKCHUNK):
        d = 2 * k + p // 64
        a = p % 64
        rows_t[k] = tmap[a, (a + d) % D]
    d32_t = tmap[np.arange(D), (np.arange(D) + 32) % D]
    return tmap, rows_t, d32_t


def _prep_layer(W, b, oper):
    """-> (wcub [NCORES](2176, oper*64) f16, wlin [NCORES](65, oper) f16)

    m = o_loc*64 + i ordering (o-major) so the i segment-sum is a 64-run."""
    _, rows_t, d32_t = _maps()
    out = W.shape[0]
    W_lin = W[:, :D]
    W_sq = W[:, D:D + 2080]
    W_cu = W[:, D + 2080:].reshape(out, D, 2080)

    iu, ju = np.triu_indices(D)
    w2 = np.zeros((out, D, D), np.float32)
    half = np.where(iu == ju, 1.0, 0.5).astype(np.float32)
    w2[:, iu, ju] = W_sq * half
    w2[:, ju, iu] = W_sq * half

    rt = rows_t.reshape(-1)
    wcubs, wlins = [], []
    for core in range(NCORES):
        o_lo = core * oper
        o_sl = [o for o in range(o_lo, o_lo + oper) if o < out]
        n_real = len(o_sl)
        M = oper * D
        wcub = np.zeros((NCHUNK * 128, M), np.float32)
        wl = np.zeros((65, oper), np.float32)
        if n_real:
            blk = W_cu[o_sl, :, :][:, :, rt]                # (n, 64, 2048)
            wcub[:2048, :n_real * D] = blk.transpose(2, 0, 1).reshape(2048, n_real * D)
            w2blk = w2[o_sl]                                # (n, 64i, 64f)
            wcub[2048:2048 + D, :n_real * D] = w2blk.transpose(2, 0, 1).reshape(D, n_real * D)
            d32blk = W_cu[o_sl, :, :][:, :, d32_t] / 2      # (n, 64i, 64a)
            wcub[2048 + D:, :n_real * D] = d32blk.transpose(2, 0, 1).reshape(D, n_real * D)
            wl[:D, :n_real] = W_lin[o_sl].T
            wl[D, :n_real] = b[o_sl]
        wcubs.append(np.ascontiguousarray(wcub.astype(np.float16)))
        wlins.append(wl.astype(np.float16))
    return wcubs, wlins


def _seg_consts():
    """(128, 32) fp16: S[p, 10*s + p//64] = 1 (cols [8s:8s+8] = slice s).

    lhsT for the i segment-sum: slice s maps tmp_s partitions q*64+i to y
    rows {2s, 2s+1}.  Columns 0:2 double as the layer-2 (oper=2) matrix."""
    S = np.zeros((128, 32), np.float32)
    for p in range(128):
        for s in range(4):
            S[p, 8 * s + 2 * s + p // 64] = 1.0
    return S.astype(np.float16)


def _in_maps(x, Ws, bs):
    """Build the per-core input maps (shared by kernel() and test.py)."""
    xT16 = np.ascontiguousarray(np.asarray(x, np.float32).astype(np.float16).T)
    wcubs, wlins = {}, {}
    for li in range(3):
        wcubs[li], wlins[li] = _prep_layer(Ws[li], bs[li], OPER[li])

    in_maps = []
    for core in range(NCORES):
        m = {"x": xT16}
        for li in range(3):
            m[f"wcub{li}"] = wcubs[li][core]
            m[f"wlin{li}"] = wlins[li][core]
        in_maps.append(m)
    return in_maps


# ------------------------------------------------------------------ builder --

def _build_module():
    import concourse.bacc as bacc
    import concourse.mybir as mybir
    import concourse.tile as tile

    F32 = mybir.dt.float32
    F16 = mybir.dt.float16
    BYPASS = mybir.AluOpType.bypass

    nc = bacc.Bacc("TRN2", target_bir_lowering=False, num_devices=NCORES, debug=False)

    x_in = nc.dram_tensor("x", [D, B], F16, kind="ExternalInput")
    wcub_in = [
        nc.dram_tensor(f"wcub{li}", [NCHUNK * 128, OPER[li] * D], F16, kind="ExternalInput")
        for li in range(3)
    ]
    wlin_in = [
        nc.dram_tensor(f"wlin{li}", [65, OPER[li]], F16, kind="ExternalInput")
        for li in range(3)
    ]
    out_ext = nc.dram_tensor("out", [OPER[2], B], F32, kind="ExternalOutput")

    seg_c = nc.inline_tensor(_seg_consts(), name="segc")

    with tile.TileContext(nc) as tc:
        with (
            tc.tile_pool(name="wpool", bufs=1) as wpool,
            tc.tile_pool(name="spool", bufs=1) as spool,
            tc.tile_pool(name="xpool", bufs=2) as xpool,
            tc.tile_pool(name="xwpool", bufs=1) as xwpool,
            tc.tile_pool(name="qpool", bufs=1) as qpool,
            tc.tile_pool(name="tpool", bufs=3) as tpool,
            tc.tile_pool(name="ypool", bufs=2) as ypool,
            tc.tile_pool(name="ps_h", bufs=4, space="PSUM") as ps_h,
            tc.tile_pool(name="ps_y", bufs=2, space="PSUM") as ps_y,
            tc.tile_pool(name="ps_w", bufs=1, space="PSUM") as ps_w,
            tc.tile_pool(name="dpool", bufs=2, space="DRAM") as dpool,
        ):
            seg_sb = spool.tile([128, 32], F16, tag="seg")
            nc.scalar.dma_start(seg_sb[:], seg_c.ap())

            # HAM keep-alive: dependency-free matmuls fill PE idle windows
            # (startup + layer boundaries) so the clock gate stays at 2.4GHz
            scr_sb = spool.tile([128, HB], F16, tag="scr")
            nc.vector.memset(scr_sb[:], 0.0)
            wps = ps_w.tile([32, HB], F32, tag="wp")
            warm_state = {"first": True}

            def warm_pe(n):
                for _ in range(n):
                    nc.tensor.matmul(
                        wps[:], seg_sb[:], scr_sb[:],
                        start=warm_state["first"], stop=False,
                        skip_group_check=True,
                    )
                    warm_state["first"] = False

            # warm the collective path early: a tiny AllGather absorbs
            # launch skew + first-call latency under layer-0 compute
            warm_in = dpool.tile([8, 8], F16, tag="warmi")
            warm_out = dpool.tile([64, 8], F16, tag="warmo")
            warm_sb = ypool.tile([8, 8], F16, tag="warm")
            nc.vector.memset(warm_sb[:], 0.0)
            nc.sync.dma_start(warm_in[:], warm_sb[:])
            nc.gpsimd.collective_compute(
                "AllGather", BYPASS, replica_groups=[list(range(NCORES))],
                ins=[warm_in.opt()], outs=[warm_out.opt()],
            )

            # per-layer weight tiles, all resident, on the ACT DMA ring;
            # layer 0 arrives slice-by-slice so its first GEMM starts sooner
            weights = []
            for li in range(3):
                M = OPER[li] * D
                wcub_sb = wpool.tile([128, NCHUNK, M], F16, tag=f"wcub{li}")
                wsrc = wcub_in[li].ap().rearrange("(k p) m -> p k m", p=128)
                if li == 0:
                    for s4 in range(4):
                        nc.scalar.dma_start(
                            wcub_sb[:, :, 128 * s4:128 * (s4 + 1)],
                            wsrc[:, :, 128 * s4:128 * (s4 + 1)],
                        )
                else:
                    nc.scalar.dma_start(wcub_sb[:], wsrc)
                wlin_sb = wpool.tile([65, OPER[li]], F16, tag=f"wlin{li}")
                nc.scalar.dma_start(wlin_sb[:], wlin_in[li].ap())
                weights.append((wcub_sb, wlin_sb))

            # x arrives pre-transposed: (64, B) in DRAM
            xsrc0 = [x_in.ap()[:, 0:HB], x_in.ap()[:, HB:B]]
            # half 0 on the SP ring, half 1 (+weights) on the ACT ring
            eng = [nc.sync, nc.scalar]

            S = {}      # per (li, h) stage state

            def ab_stage(li, h, xsrc):
                """Phase A+B: x views, shifted copies, rotation products."""
                if li == 2:
                    # tail layer: both rings are draining, alternate per call
                    ndma = [0]

                    def dma(*a):
                        eng[ndma[0] % 2].dma_start(*a)
                        ndma[0] += 1
                else:
                    dma = eng[h].dma_start
                xt = xpool.tile([65, HB], F16, tag=f"xT{li&1}{h}")
                dma(xt[0:D, :], xsrc)
                nc.vector.memset(xt[D:65, :], 1.0)

                # xwA[p, q] = x_{(p+q)%64} for q in {0,1}
                xwA = xwpool.tile([128, 2, HB], F16, tag=f"xwA{li&1}{h}")
                dma(xwA[0:D, 0, :], xsrc)
                dma(xwA[D:128, 0, :], xsrc)
                dma(xwA[0:D - 1, 1, :], xsrc[1:D, :])
                dma(xwA[D - 1:D, 1, :], xsrc[0:1, :])
                dma(xwA[D:127, 1, :], xsrc[1:D, :])
                dma(xwA[127:128, 1, :], xsrc[0:1, :])

                # xw[p, q, j] = x_{(p+2j+q)%64} = xwA[p+2j, q]; rotation
                # windows only read rows < 88, so wrap rows are skipped
                xw = xwpool.tile([128, 2, 4, HB], F16, tag=f"xw{li&1}{h}")
                dma(xw[:, :, 0, :], xwA[:])
                for j in (1, 2, 3):
                    dma(xw[0:128 - 2 * j, :, j, :], xwA[2 * j:128, :, :])

                xstack = xpool.tile([128, HB], F16, tag=f"xstack{li&1}{h}")
                dma(xstack[0:D, :], xwA[0:D, 0, :])
                # gap-32 products (halved in the weights)
                rot32 = xpool.tile([D, HB], F16, tag=f"rot32{li&1}{h}")
                dma(rot32[:], xwA[32:96, 0, :])
                nc.vector.tensor_mul(xstack[D:128, :], xwA[0:D, 0, :], rot32[:])

                # rot[q*64+a, j, c] = xw[2k0+a, q, j, c]: chunks k0..k0+3
                x2b = xw[:, 0, 0, :].unsqueeze(1).broadcast_to([128, 4, HB])
                xsq = []
                for k0 in (0, 4, 8, 12):
                    rot = qpool.tile([128, 4, HB], F16, tag=f"rot{k0}h{h}")
                    dma(rot[0:D, :, :], xw[2 * k0:2 * k0 + D, 0, :, :])
                    dma(rot[D:128, :, :], xw[2 * k0:2 * k0 + D, 1, :, :])
                    xq = qpool.tile([128, 4, HB], F16, tag=f"xsq{k0}h{h}")
                    nc.vector.tensor_mul(xq[:], x2b, rot[:])
                    for j in range(4):
                        xsq.append(xq[:, j, :])
                xsq.append(xstack[:])
                S[(li, h)] = (xt, xw, xsq)

            def c_stage(li, h):
                """Phase C + segment-sum + boundary (AllGather or store).

                Returns the next layer's x source AP (or None for layer 2)."""
                oper = OPER[li]
                nsl = (oper * D) // 128          # m slices: 4, 4, 1
                last = li == 2
                wcub_sb, wlin_sb = weights[li]
                xt, xw, xsq = S[(li, h)]

                y_ps = ps_y.tile([oper, HB], F32, tag="y")
                nc.tensor.matmul(
                    y_ps[:], wlin_sb[:], xt[:],
                    start=True, stop=False, skip_group_check=True,
                )
                for s in range(nsl):
                    h_ps = ps_h.tile([128, HB], F32, tag="h")
                    for k in range(NCHUNK):
                        nc.tensor.matmul(
                            h_ps[:], wcub_sb[:, k, 128 * s:128 * (s + 1)],
                            xsq[k], start=(k == 0), stop=(k == NCHUNK - 1),
                        )
                    tmp = tpool.tile([128, HB], F16, tag="tmp")
                    nc.vector.tensor_mul(tmp[:], h_ps[:], xw[:, 0, 0, :])
                    nc.tensor.matmul(
                        y_ps[:], seg_sb[:, 8 * s:8 * s + oper],
                        tmp[:], start=False, stop=(s == nsl - 1),
                        skip_group_check=True,
                    )

                if not last:
                    y_sb = ypool.tile([oper, HB], F16, tag=f"y{h}")
                    nc.scalar.copy(y_sb[:], y_ps[:])
                    y_bounce = dpool.tile([oper, HB], F16, tag=f"yb{li}{h}")
                    yg = dpool.tile([D, HB], F16, tag=f"yg{li}{h}")
                    eng[h].dma_start(y_bounce[:], y_sb[:])
                    nc.gpsimd.collective_compute(
                        "AllGather",
                        BYPASS,
                        replica_groups=[list(range(NCORES))],
                        ins=[y_bounce.opt()],
                        outs=[yg.opt()],
                    )
                    return yg[:]
                y_sb = ypool.tile([oper, HB], F32, tag=f"yf{h}")
                nc.scalar.copy(y_sb[:], y_ps[:])
                eng[h].dma_start(out_ext.ap()[:, h * HB:(h + 1) * HB], y_sb[:])
                return None

            # software-pipelined emission: AB(li+1, h) is issued right after
            # C(li, h) so its DMA/DVE chain hides under C(li, 1-h) on the PE
            warm_pe(56)
            ab_stage(0, 0, xsrc0[0])
            ab_stage(0, 1, xsrc0[1])
            nxt = [None, None]
            for li in range(3):
                for h in range(2):
                    nxt[h] = c_stage(li, h)
                    if li < 2:
                        ab_stage(li + 1, h, nxt[h])
                    if not (li == 2 and h == 1):
                        warm_pe(20)
            # keep the warm accumulator live so DCE cannot drop the blocks
            nc.tensor.matmul(
                wps[:], seg_sb[:], scr_sb[:],
                start=False, stop=True, skip_group_check=True,
            )
            dead_sb = ypool.tile([8, 8], F16, tag="dead")
            nc.scalar.copy(dead_sb[:], wps[0:8, 0:8])
            nc.sync.dma_start(warm_in[:], dead_sb[:])

    nc.compile()
    return nc


# ------------------------------------------------------------------- runner --

def kernel(x, W0, b0, W1, b1, W2, b2):
    from concourse.bass_utils import run_bass_kernel_spmd

    if "nc" not in _CACHE:
        _CACHE["nc"] = _build_module()
    nc = _CACHE["nc"]

    Ws = [np.asarray(W, np.float32) for W in (W0, W1, W2)]
    bs = [np.asarray(b_, np.float32) for b_ in (b0, b1, b2)]

    in_maps = _in_maps(x, Ws, bs)
    res = run_bass_kernel_spmd(nc, in_maps, core_ids=list(range(NCORES)))
    out = np.zeros((B, OUTS[2]), np.float32)
    for core in range(5):
        o_lo = core * OPER[2]
        n = min(OPER[2], OUTS[2] - o_lo)
        out[:, o_lo:o_lo + n] = res.results[core]["out"][:n, :].T
    return out
